# revision 4
# baseline (speedup 1.0000x reference)
"""Trainium2 Bass kernel for nn_CombineJaggedEmbedding (jagged embedding-bag).

Computes, for two independent features:
  out0[b] = sum_{j in [off0[b], off0[b+1])} weights0[j] * table0[idx0[j]]   [B, 64]
  out1[b] = mean_{j in [off1[b], off1[b+1])} table1[idx1[j]]               [B, 128]

Strategy (8 cores, vocab-sharded per the sharding hint):
  - Each core owns a V/8 = 25000-row slice of each table (local row ids fit
    int16, enabling the fast Q7 dma_gather/dma_scatter_add ucode path).
  - The host routes each jagged value to the core owning its table row,
    keeping values in original (segment-sorted) order per core.
  - Per core, per 128-value block: dma_gather the rows (value v sits on
    partition v%128), build a one-hot matrix M[j, s] = w[j] *
    (seg[j] - group_base == s) on the vector engine, and matmul
    psum[128, D] += M.T @ emb on the tensor engine, accumulating a
    "group" of gb*128 consecutive values whose segments span <= 127.
  - dma_scatter_add each group's [128, D] partial segment-sums into a
    per-core [B+1+ngroups, D] DRAM buffer. Rows are unique per core
    (segments straddling a group boundary go to per-group scratch rows),
    so adds into the zero-initialized buffer are race-free.
  - Host sums the 8 per-core buffers and folds the scratch rows back in
    (the cross-shard combine the hint's all-to-all would otherwise do).
  The "mean" feature uses w[j] = 1/segment_count as the per-value weight,
  making both features the same weighted-sum kernel.
"""

import numpy as np

_B = 16384
_T = 819200
_V = 200000
_D0 = 64
_D1 = 128
_NCORES = 8
_VSH = _V // _NCORES  # 25000 table rows per core
_BLK = 128
_CHUNK = 4096  # values per dma_gather (32 blocks)
_CHUNK_BLKS = _CHUNK // _BLK


def _pick_group_blks(nseg_max_fn):
    """Largest group size (in 128-value blocks) whose worst-case segment span
    (base_{g+1} - base_g) stays <= 127 so a group fits one 128-row PSUM tile."""
    for gb in (16, 8, 4, 2, 1):
        if _CHUNK_BLKS % gb == 0 and nseg_max_fn(gb) <= 127:
            return gb
    raise AssertionError("no feasible group size; offsets pathological")


def _wrap16(flat):
    """int16 index layout for dma_gather/dma_scatter_add: element i sits at
    [i % 16, i // 16], replicated across the 8 Q7 cores (16*8=128 parts)."""
    t = flat.astype(np.int16).reshape(-1, 16).T  # [16, n/16]
    return np.ascontiguousarray(np.tile(t, (_NCORES, 1)))


def _prep_feature(idx, w_eff, off):
    """Host-side preprocessing for one feature: route values to vocab shards,
    build per-core packed index/weight/one-hot-column arrays and scatter maps.
    """
    seg = np.searchsorted(off, np.arange(_T, dtype=np.int64), side="right") - 1
    shard = idx // _VSH

    j_lists = [np.nonzero(shard == k)[0] for k in range(_NCORES)]
    nmax = max(len(j) for j in j_lists)
    vt = ((nmax + _CHUNK - 1) // _CHUNK) * _CHUNK  # padded values per core
    nblk = vt // _BLK

    # pick group size from worst-case span over all cores
    def nseg_max(gb):
        gv = gb * _BLK
        m = 0
        for j in j_lists:
            s = seg[j]
            s = np.append(s, np.full(vt - len(s), s[-1] if len(s) else 0))
            base = s[::gv]
            base_next = np.append(base[1:], s[-1] + 1)
            m = max(m, int((base_next - base).max()))
        return m

    gb = _pick_group_blks(nseg_max)
    gv = gb * _BLK
    ngrp = vt // gv

    per_core = []
    combine = []
    for k in range(_NCORES):
        j = j_lists[k]
        n = len(j)
        seg_k = seg[j]
        pad_seg = seg_k[-1] if n else 0
        seg_p = np.append(seg_k, np.full(vt - n, pad_seg))
        idx_p = np.append(idx[j] - k * _VSH, np.zeros(vt - n, np.int64))
        w_p = np.append(w_eff[j], np.zeros(vt - n, np.float32)).astype(np.float32)

        base = seg_p[::gv]  # [ngrp]
        base_next = np.append(base[1:], seg_p[-1] + 1)
        nseg = base_next - base
        assert int(nseg.max()) <= 127

        col = (seg_p - np.repeat(base, gv)).astype(np.float32)
        assert col.min() >= 0.0 and col.max() <= 127.0

        # scatter rows [128, ngrp]: psum row p of group g -> DRAM row
        p = np.arange(_BLK, dtype=np.int64)[:, None]
        rows = base[None, :] + p
        rows = np.where(p == nseg[None, :], _B + 1 + np.arange(ngrp)[None, :], rows)
        rows = np.where(p > nseg[None, :], _B, rows)  # trash row

        def pack(a):
            # value v = b*128 + p  ->  packed[p, b]
            return np.ascontiguousarray(a.reshape(-1, _BLK).T)

        per_core.append(
            {
                "gidx": _wrap16(idx_p),  # [128, vt/16] i16
                "col": pack(col).astype(np.float32),  # [128, nblk] f32
                "w": pack(w_p),  # [128, nblk] f32
                "srow": _wrap16(rows.T.reshape(-1)),  # [128, ngrp*8] i16
            }
        )
        combine.append(
            {
                "lo": int(seg_k[0]) if n else 0,
                "hi": (int(seg_k[-1]) + 1) if n else 0,
                "base_next": base_next,
            }
        )
    return dict(gb=gb, vt=vt, nblk=nblk, ngrp=ngrp, per_core=per_core,
                combine=combine)


_NC_CACHE = {}


def _build_nc(cfg0, cfg1):
    """Build (and cache) the SPMD Bass/Tile program; all data-dependence is
    carried by input tensors, so one program serves all 8 cores."""
    key = (cfg0["gb"], cfg0["vt"], cfg1["gb"], cfg1["vt"])
    if key in _NC_CACHE:
        return _NC_CACHE[key]

    from contextlib import ExitStack

    import concourse.tile as tile
    from concourse import bacc, mybir

    f32 = mybir.dt.float32
    i16 = mybir.dt.int16

    nc = bacc.Bacc(
        "TRN2", target_bir_lowering=False, debug=False, enable_asserts=False
    )

    feats = []
    for fi, (dd, cfg) in enumerate(((_D0, cfg0), (_D1, cfg1))):
        vt, nblk, ngrp = cfg["vt"], cfg["nblk"], cfg["ngrp"]
        feats.append(
            dict(
                fi=fi,
                D=dd,
                gb=cfg["gb"],
                ngrp=ngrp,
                nblk=nblk,
                vt=vt,
                tab=nc.dram_tensor(f"table{fi}", [_VSH, dd], f32, kind="ExternalInput"),
                gidx=nc.dram_tensor(f"gidx{fi}", [_BLK, vt // 16], i16, kind="ExternalInput"),
                col=nc.dram_tensor(f"col{fi}", [_BLK, nblk], f32, kind="ExternalInput"),
                w=nc.dram_tensor(f"w{fi}", [_BLK, nblk], f32, kind="ExternalInput"),
                srow=nc.dram_tensor(f"srow{fi}", [_BLK, ngrp * 8], i16, kind="ExternalInput"),
                out=nc.dram_tensor(f"out{fi}", [_B + 1 + ngrp, dd], f32, kind="ExternalOutput"),
            )
        )

    iota_np = np.tile(np.arange(_BLK, dtype=np.float32), (_BLK, 1))
    iota_dram = nc.inline_tensor(iota_np, name="iota128")

    with tile.TileContext(nc) as tc, ExitStack() as ctx:
        cpool = ctx.enter_context(tc.tile_pool(name="consts", bufs=1))
        iota_t = cpool.tile([_BLK, _BLK], f32, tag="iota")
        nc.sync.dma_start(iota_t[:], iota_dram.ap())

        for f in feats:
            D, gb, ngrp, nblk, vt = f["D"], f["gb"], f["ngrp"], f["nblk"], f["vt"]
            gpc = _CHUNK_BLKS // gb  # groups per chunk
            nchunk = vt // _CHUNK

            gidx_t = cpool.tile([_BLK, vt // 16], i16, tag=f"gidx{f['fi']}")
            col_t = cpool.tile([_BLK, nblk], f32, tag=f"col{f['fi']}")
            w_t = cpool.tile([_BLK, nblk], f32, tag=f"w{f['fi']}")
            srow_t = cpool.tile([_BLK, ngrp * 8], i16, tag=f"srow{f['fi']}")
            nc.sync.dma_start(gidx_t[:], f["gidx"].ap())
            nc.sync.dma_start(col_t[:], f["col"].ap())
            nc.sync.dma_start(w_t[:], f["w"].ap())
            nc.sync.dma_start(srow_t[:], f["srow"].ap())

            epool = ctx.enter_context(tc.tile_pool(name=f"emb{f['fi']}", bufs=3))
            spool = ctx.enter_context(tc.tile_pool(name=f"st{f['fi']}", bufs=3))
            mpool = ctx.enter_context(tc.tile_pool(name=f"m{f['fi']}", bufs=4))
            pspool = ctx.enter_context(
                tc.tile_pool(name=f"ps{f['fi']}", bufs=4, space="PSUM")
            )

            for c in range(nchunk):
                emb = epool.tile([_BLK, _CHUNK_BLKS * D], f32, tag="emb")
                nc.gpsimd.dma_gather(
                    out_ap=emb[:].rearrange("p (k d) -> p k d", d=D),
                    in_ap=f["tab"].ap(),
                    idxs_ap=gidx_t[:, c * (_CHUNK // 16) : (c + 1) * (_CHUNK // 16)],
                    num_idxs=_CHUNK,
                    num_idxs_reg=_CHUNK,
                    elem_size=D,
                    # >1024 descriptors don't fit one SWDGE packet; leaving
                    # single_packet on crashes the exec unit at this size
                    single_packet=False,
                )
                staged = spool.tile([_BLK, gpc * D], f32, tag="staged")
                for gi in range(gpc):
                    g = c * gpc + gi
                    ps = pspool.tile([_BLK, D], f32, space="PSUM", tag="ps")
                    for b in range(gb):
                        blk = gi * gb + b
                        gblk = c * _CHUNK_BLKS + blk
                        m = mpool.tile([_BLK, _BLK], f32, tag="m")
                        nc.vector.tensor_scalar(
                            out=m[:],
                            in0=iota_t[:],
                            scalar1=col_t[:, gblk : gblk + 1],
                            scalar2=w_t[:, gblk : gblk + 1],
                            op0=mybir.AluOpType.is_equal,
                            op1=mybir.AluOpType.mult,
                        )
                        nc.tensor.matmul(
                            out=ps[:],
                            lhsT=m[:],
                            rhs=emb[:, blk * D : (blk + 1) * D],
                            start=(b == 0),
                            stop=(b == gb - 1),
                        )
                    nc.scalar.copy(staged[:, gi * D : (gi + 1) * D], ps[:])

                nc.gpsimd.dma_scatter_add(
                    out_ap=f["out"].ap(),
                    in_ap=staged[:].rearrange("p (k d) -> p k d", d=D),
                    idxs_ap=srow_t[:, c * gpc * 8 : (c + 1) * gpc * 8],
                    num_idxs=gpc * _BLK,
                    num_idxs_reg=gpc * _BLK,
                    elem_size=D,
                )

    nc.compile()
    _NC_CACHE[key] = nc
    return nc


_LAST_RESULT = None  # BassKernelResults of the most recent run (for test.py)


def kernel(table0, table1, weights0, idx0, idx1, off0, off1, _trace=False):
    global _LAST_RESULT

    table0 = np.ascontiguousarray(np.asarray(table0, dtype=np.float32))
    table1 = np.ascontiguousarray(np.asarray(table1, dtype=np.float32))
    weights0 = np.asarray(weights0, dtype=np.float32)
    idx0 = np.asarray(idx0, dtype=np.int64)
    idx1 = np.asarray(idx1, dtype=np.int64)
    off0 = np.asarray(off0, dtype=np.int64)
    off1 = np.asarray(off1, dtype=np.int64)

    # Feature 1 is a mean-pool: fold 1/count into per-value weights.
    counts1 = (off1[1:] - off1[:-1]).astype(np.float64)
    seg1 = np.searchsorted(off1, np.arange(_T, dtype=np.int64), side="right") - 1
    w1 = (1.0 / np.maximum(counts1[seg1], 1.0)).astype(np.float32)

    cfg0 = _prep_feature(idx0, weights0, off0)
    cfg1 = _prep_feature(idx1, w1, off1)

    nc = _build_nc(cfg0, cfg1)

    in_maps = []
    for k in range(_NCORES):
        m = {
            "table0": table0[k * _VSH : (k + 1) * _VSH],
            "table1": table1[k * _VSH : (k + 1) * _VSH],
        }
        for fi, cfg in ((0, cfg0), (1, cfg1)):
            pc = cfg["per_core"][k]
            m[f"gidx{fi}"] = pc["gidx"]
            m[f"col{fi}"] = pc["col"]
            m[f"w{fi}"] = pc["w"]
            m[f"srow{fi}"] = pc["srow"]
        in_maps.append(m)

    from concourse.bass_utils import run_bass_kernel_spmd

    res = run_bass_kernel_spmd(
        nc, in_maps, core_ids=list(range(_NCORES)), trace=_trace
    )
    _LAST_RESULT = res

    out0 = np.zeros((_B, _D0), dtype=np.float32)
    out1 = np.zeros((_B, _D1), dtype=np.float32)
    for out, cfg, name in ((out0, cfg0, "out0"), (out1, cfg1, "out1")):
        ngrp = cfg["ngrp"]
        for k in range(_NCORES):
            buf = res.results[k][name]
            cmb = cfg["combine"][k]
            lo, hi = cmb["lo"], cmb["hi"]
            out[lo:hi] += buf[lo:hi]
            bn = cmb["base_next"]
            valid = bn < _B
            np.add.at(out, bn[valid], buf[_B + 1 : _B + 1 + ngrp][valid])
    return out0, out1


# revision 6
# speedup vs baseline: 1.1048x; 1.1048x over previous
"""Trainium2 Bass kernel for nn_CombineJaggedEmbedding (jagged embedding-bag).

Computes, for two independent features:
  out0[b] = sum_{j in [off0[b], off0[b+1])} weights0[j] * table0[idx0[j]]   [B, 64]
  out1[b] = mean_{j in [off1[b], off1[b+1])} table1[idx1[j]]               [B, 128]

Strategy (8 cores, vocab-sharded per the sharding hint):
  - Each core owns a V/8 = 25000-row slice of each table (local row ids fit
    int16, enabling the fast Q7 dma_gather/dma_scatter_add ucode path).
  - The host routes each jagged value to the core owning its table row,
    keeping values in original (segment-sorted) order per core.
  - Per core, per 128-value block: dma_gather the rows (value v sits on
    partition v%128), build a one-hot matrix M[j, s] = w[j] *
    (seg[j] - group_base == s) on the vector engine, and matmul
    psum[128, D] += M.T @ emb on the tensor engine, accumulating a
    "group" of gb*128 consecutive values whose segments span <= 127.
  - dma_scatter_add each group's [128, D] partial segment-sums into a
    per-core [B+1+ngroups, D] DRAM buffer. Rows are unique per core
    (segments straddling a group boundary go to per-group scratch rows),
    so adds into the zero-initialized buffer are race-free.
  - Host sums the 8 per-core buffers and folds the scratch rows back in
    (the cross-shard combine the hint's all-to-all would otherwise do).
  The "mean" feature uses w[j] = 1/segment_count as the per-value weight,
  making both features the same weighted-sum kernel.
"""

import numpy as np

_B = 16384
_T = 819200
_V = 200000
_D0 = 64
_D1 = 128
_NCORES = 8
_VSH = _V // _NCORES  # 25000 table rows per core
_BLK = 128
_CHUNK = 4096  # values per dma_gather (32 blocks)
_CHUNK_BLKS = _CHUNK // _BLK


def _pick_group_blks(nseg_max_fn):
    """Largest group size (in 128-value blocks) whose worst-case segment span
    (base_{g+1} - base_g) stays <= 127 so a group fits one 128-row PSUM tile."""
    for gb in (16, 8, 4, 2, 1):
        if _CHUNK_BLKS % gb == 0 and nseg_max_fn(gb) <= 127:
            return gb
    raise AssertionError("no feasible group size; offsets pathological")


def _wrap16(flat):
    """int16 index layout for dma_gather/dma_scatter_add: element i sits at
    [i % 16, i // 16], replicated across the 8 Q7 cores (16*8=128 parts)."""
    t = flat.astype(np.int16).reshape(-1, 16).T  # [16, n/16]
    return np.ascontiguousarray(np.tile(t, (_NCORES, 1)))


def _prep_feature(idx, w_eff, off):
    """Host-side preprocessing for one feature: route values to vocab shards,
    build per-core packed index/weight/one-hot-column arrays and scatter maps.
    """
    seg = np.searchsorted(off, np.arange(_T, dtype=np.int64), side="right") - 1
    shard = idx // _VSH

    j_lists = [np.nonzero(shard == k)[0] for k in range(_NCORES)]
    nmax = max(len(j) for j in j_lists)
    vt = ((nmax + _CHUNK - 1) // _CHUNK) * _CHUNK  # padded values per core
    nblk = vt // _BLK

    # pick group size from worst-case span over all cores
    def nseg_max(gb):
        gv = gb * _BLK
        m = 0
        for j in j_lists:
            s = seg[j]
            s = np.append(s, np.full(vt - len(s), s[-1] if len(s) else 0))
            base = s[::gv]
            base_next = np.append(base[1:], s[-1] + 1)
            m = max(m, int((base_next - base).max()))
        return m

    gb = _pick_group_blks(nseg_max)
    gv = gb * _BLK
    ngrp = vt // gv

    per_core = []
    combine = []
    for k in range(_NCORES):
        j = j_lists[k]
        n = len(j)
        seg_k = seg[j]
        pad_seg = seg_k[-1] if n else 0
        seg_p = np.append(seg_k, np.full(vt - n, pad_seg))
        idx_p = np.append(idx[j] - k * _VSH, np.zeros(vt - n, np.int64))
        w_p = np.append(w_eff[j], np.zeros(vt - n, np.float32)).astype(np.float32)

        base = seg_p[::gv]  # [ngrp]
        base_next = np.append(base[1:], seg_p[-1] + 1)
        nseg = base_next - base
        assert int(nseg.max()) <= 127

        col = (seg_p - np.repeat(base, gv)).astype(np.float32)
        assert col.min() >= 0.0 and col.max() <= 127.0

        # scatter rows [128, ngrp]: psum row p of group g -> DRAM row
        p = np.arange(_BLK, dtype=np.int64)[:, None]
        rows = base[None, :] + p
        rows = np.where(p == nseg[None, :], _B + 1 + np.arange(ngrp)[None, :], rows)
        rows = np.where(p > nseg[None, :], _B, rows)  # trash row

        def pack(a):
            # value v = b*128 + p  ->  packed[p, b]
            return np.ascontiguousarray(a.reshape(-1, _BLK).T)

        per_core.append(
            {
                "gidx": _wrap16(idx_p),  # [128, vt/16] i16
                "col": pack(col).astype(np.float32),  # [128, nblk] f32
                "w": pack(w_p),  # [128, nblk] f32
                "srow": _wrap16(rows.T.reshape(-1)),  # [128, ngrp*8] i16
            }
        )
        combine.append(
            {
                "lo": int(seg_k[0]) if n else 0,
                "hi": (int(seg_k[-1]) + 1) if n else 0,
                "base_next": base_next,
            }
        )
    return dict(gb=gb, vt=vt, nblk=nblk, ngrp=ngrp, per_core=per_core,
                combine=combine)


_NC_CACHE = {}


def _build_nc(cfg0, cfg1):
    """Build (and cache) the SPMD Bass/Tile program; all data-dependence is
    carried by input tensors, so one program serves all 8 cores."""
    key = (cfg0["gb"], cfg0["vt"], cfg1["gb"], cfg1["vt"])
    if key in _NC_CACHE:
        return _NC_CACHE[key]

    from contextlib import ExitStack

    import concourse.tile as tile
    from concourse import bacc, mybir

    f32 = mybir.dt.float32
    i16 = mybir.dt.int16

    nc = bacc.Bacc(
        "TRN2", target_bir_lowering=False, debug=False, enable_asserts=False
    )

    feats = []
    for fi, (dd, cfg) in enumerate(((_D0, cfg0), (_D1, cfg1))):
        vt, nblk, ngrp = cfg["vt"], cfg["nblk"], cfg["ngrp"]
        feats.append(
            dict(
                fi=fi,
                D=dd,
                gb=cfg["gb"],
                ngrp=ngrp,
                nblk=nblk,
                vt=vt,
                tab=nc.dram_tensor(f"table{fi}", [_VSH, dd], f32, kind="ExternalInput"),
                gidx=nc.dram_tensor(f"gidx{fi}", [_BLK, vt // 16], i16, kind="ExternalInput"),
                col=nc.dram_tensor(f"col{fi}", [_BLK, nblk], f32, kind="ExternalInput"),
                w=nc.dram_tensor(f"w{fi}", [_BLK, nblk], f32, kind="ExternalInput"),
                srow=nc.dram_tensor(f"srow{fi}", [_BLK, ngrp * 8], i16, kind="ExternalInput"),
                out=nc.dram_tensor(f"out{fi}", [_B + 1 + ngrp, dd], f32, kind="ExternalOutput"),
            )
        )

    iota_np = np.tile(np.arange(_BLK, dtype=np.float32), (_BLK, 1))
    iota_dram = nc.inline_tensor(iota_np, name="iota128")

    with tile.TileContext(nc) as tc, ExitStack() as ctx:
        cpool = ctx.enter_context(tc.tile_pool(name="consts", bufs=1))
        iota_t = cpool.tile([_BLK, _BLK], f32, tag="iota")
        nc.sync.dma_start(iota_t[:], iota_dram.ap())

        for f in feats:
            D, gb, ngrp, nblk, vt = f["D"], f["gb"], f["ngrp"], f["nblk"], f["vt"]
            gpc = _CHUNK_BLKS // gb  # groups per chunk
            nchunk = vt // _CHUNK

            gidx_t = cpool.tile([_BLK, vt // 16], i16, tag=f"gidx{f['fi']}")
            col_t = cpool.tile([_BLK, nblk], f32, tag=f"col{f['fi']}")
            w_t = cpool.tile([_BLK, nblk], f32, tag=f"w{f['fi']}")
            srow_t = cpool.tile([_BLK, ngrp * 8], i16, tag=f"srow{f['fi']}")
            nc.sync.dma_start(gidx_t[:], f["gidx"].ap())
            nc.sync.dma_start(col_t[:], f["col"].ap())
            nc.sync.dma_start(w_t[:], f["w"].ap())
            nc.sync.dma_start(srow_t[:], f["srow"].ap())

            epool = ctx.enter_context(tc.tile_pool(name=f"emb{f['fi']}", bufs=3))
            spool = ctx.enter_context(tc.tile_pool(name=f"st{f['fi']}", bufs=3))
            mpool = ctx.enter_context(tc.tile_pool(name=f"m{f['fi']}", bufs=2))
            pspool = ctx.enter_context(
                tc.tile_pool(name=f"ps{f['fi']}", bufs=4, space="PSUM")
            )

            for c in range(nchunk):
                emb = epool.tile([_BLK, _CHUNK_BLKS * D], f32, tag="emb")
                nc.gpsimd.dma_gather(
                    out_ap=emb[:].rearrange("p (k d) -> p k d", d=D),
                    in_ap=f["tab"].ap(),
                    idxs_ap=gidx_t[:, c * (_CHUNK // 16) : (c + 1) * (_CHUNK // 16)],
                    num_idxs=_CHUNK,
                    num_idxs_reg=_CHUNK,
                    elem_size=D,
                    # >1024 descriptors don't fit one SWDGE packet; leaving
                    # single_packet on crashes the exec unit at this size
                    single_packet=False,
                )
                # fold the per-value weight into the gathered rows (in place)
                nc.vector.tensor_tensor(
                    out=emb[:].rearrange("p (k d) -> p k d", d=D),
                    in0=emb[:].rearrange("p (k d) -> p k d", d=D),
                    in1=w_t[:, c * _CHUNK_BLKS : (c + 1) * _CHUNK_BLKS]
                    .rearrange("p (k one) -> p k one", one=1)
                    .to_broadcast([_BLK, _CHUNK_BLKS, D]),
                    op=mybir.AluOpType.mult,
                )
                # one-hot block-column matrices for the whole chunk in one op:
                # m[j, (k, s)] = (col[j, k] == s)
                m = mpool.tile([_BLK, _CHUNK_BLKS * _BLK], f32, tag="m")
                nc.vector.tensor_tensor(
                    out=m[:].rearrange("p (k s) -> p k s", s=_BLK),
                    in0=iota_t[:]
                    .rearrange("p (one s) -> p one s", one=1)
                    .to_broadcast([_BLK, _CHUNK_BLKS, _BLK]),
                    in1=col_t[:, c * _CHUNK_BLKS : (c + 1) * _CHUNK_BLKS]
                    .rearrange("p (k one) -> p k one", one=1)
                    .to_broadcast([_BLK, _CHUNK_BLKS, _BLK]),
                    op=mybir.AluOpType.is_equal,
                )
                staged = spool.tile([_BLK, gpc * D], f32, tag="staged")
                for gi in range(gpc):
                    g = c * gpc + gi
                    ps = pspool.tile([_BLK, D], f32, space="PSUM", tag="ps")
                    for b in range(gb):
                        blk = gi * gb + b
                        nc.tensor.matmul(
                            out=ps[:],
                            lhsT=m[:, blk * _BLK : (blk + 1) * _BLK],
                            rhs=emb[:, blk * D : (blk + 1) * D],
                            start=(b == 0),
                            stop=(b == gb - 1),
                        )
                    nc.scalar.copy(staged[:, gi * D : (gi + 1) * D], ps[:])

                nc.gpsimd.dma_scatter_add(
                    out_ap=f["out"].ap(),
                    in_ap=staged[:].rearrange("p (k d) -> p k d", d=D),
                    idxs_ap=srow_t[:, c * gpc * 8 : (c + 1) * gpc * 8],
                    num_idxs=gpc * _BLK,
                    num_idxs_reg=gpc * _BLK,
                    elem_size=D,
                )

    nc.compile()
    _NC_CACHE[key] = nc
    return nc


_LAST_RESULT = None  # BassKernelResults of the most recent run (for test.py)


def kernel(table0, table1, weights0, idx0, idx1, off0, off1, _trace=False):
    global _LAST_RESULT

    table0 = np.ascontiguousarray(np.asarray(table0, dtype=np.float32))
    table1 = np.ascontiguousarray(np.asarray(table1, dtype=np.float32))
    weights0 = np.asarray(weights0, dtype=np.float32)
    idx0 = np.asarray(idx0, dtype=np.int64)
    idx1 = np.asarray(idx1, dtype=np.int64)
    off0 = np.asarray(off0, dtype=np.int64)
    off1 = np.asarray(off1, dtype=np.int64)

    # Feature 1 is a mean-pool: fold 1/count into per-value weights.
    counts1 = (off1[1:] - off1[:-1]).astype(np.float64)
    seg1 = np.searchsorted(off1, np.arange(_T, dtype=np.int64), side="right") - 1
    w1 = (1.0 / np.maximum(counts1[seg1], 1.0)).astype(np.float32)

    cfg0 = _prep_feature(idx0, weights0, off0)
    cfg1 = _prep_feature(idx1, w1, off1)

    nc = _build_nc(cfg0, cfg1)

    in_maps = []
    for k in range(_NCORES):
        m = {
            "table0": table0[k * _VSH : (k + 1) * _VSH],
            "table1": table1[k * _VSH : (k + 1) * _VSH],
        }
        for fi, cfg in ((0, cfg0), (1, cfg1)):
            pc = cfg["per_core"][k]
            m[f"gidx{fi}"] = pc["gidx"]
            m[f"col{fi}"] = pc["col"]
            m[f"w{fi}"] = pc["w"]
            m[f"srow{fi}"] = pc["srow"]
        in_maps.append(m)

    from concourse.bass_utils import run_bass_kernel_spmd

    res = run_bass_kernel_spmd(
        nc, in_maps, core_ids=list(range(_NCORES)), trace=_trace
    )
    _LAST_RESULT = res

    out0 = np.zeros((_B, _D0), dtype=np.float32)
    out1 = np.zeros((_B, _D1), dtype=np.float32)
    for out, cfg, name in ((out0, cfg0, "out0"), (out1, cfg1, "out1")):
        ngrp = cfg["ngrp"]
        for k in range(_NCORES):
            buf = res.results[k][name]
            cmb = cfg["combine"][k]
            lo, hi = cmb["lo"], cmb["hi"]
            out[lo:hi] += buf[lo:hi]
            bn = cmb["base_next"]
            valid = bn < _B
            np.add.at(out, bn[valid], buf[_B + 1 : _B + 1 + ngrp][valid])
    return out0, out1


# revision 8
# speedup vs baseline: 1.1404x; 1.0322x over previous
"""Trainium2 Bass kernel for nn_CombineJaggedEmbedding (jagged embedding-bag).

Computes, for two independent features:
  out0[b] = sum_{j in [off0[b], off0[b+1])} weights0[j] * table0[idx0[j]]   [B, 64]
  out1[b] = mean_{j in [off1[b], off1[b+1])} table1[idx1[j]]               [B, 128]

Strategy (8 cores, vocab-sharded per the sharding hint):
  - Each core owns a V/8 = 25000-row slice of each table (local row ids fit
    int16, enabling the fast Q7 dma_gather/dma_scatter_add ucode path).
  - The host routes each jagged value to the core owning its table row,
    keeping values in original (segment-sorted) order per core.
  - Per core, per 128-value block: dma_gather the rows (value v sits on
    partition v%128), build a one-hot matrix M[j, s] = w[j] *
    (seg[j] - group_base == s) on the vector engine, and matmul
    psum[128, D] += M.T @ emb on the tensor engine, accumulating a
    "group" of gb*128 consecutive values whose segments span <= 127.
  - dma_scatter_add each group's [128, D] partial segment-sums into a
    per-core [B+1+ngroups, D] DRAM buffer. Rows are unique per core
    (segments straddling a group boundary go to per-group scratch rows),
    so adds into the zero-initialized buffer are race-free.
  - Host sums the 8 per-core buffers and folds the scratch rows back in
    (the cross-shard combine the hint's all-to-all would otherwise do).
  The "mean" feature uses w[j] = 1/segment_count as the per-value weight,
  making both features the same weighted-sum kernel.
"""

import numpy as np

_B = 16384
_T = 819200
_V = 200000
_D0 = 64
_D1 = 128
_NCORES = 8
_VSH = _V // _NCORES  # 25000 table rows per core
_BLK = 128
_CHUNK = 4096  # values per dma_gather (32 blocks)
_CHUNK_BLKS = _CHUNK // _BLK


def _pick_group_blks(nseg_max_fn):
    """Largest group size (in 128-value blocks) whose worst-case segment span
    (base_{g+1} - base_g) stays <= 127 so a group fits one 128-row PSUM tile."""
    for gb in (16, 8, 4, 2, 1):
        if _CHUNK_BLKS % gb == 0 and nseg_max_fn(gb) <= 127:
            return gb
    raise AssertionError("no feasible group size; offsets pathological")


def _wrap16(flat):
    """int16 index layout for dma_gather/dma_scatter_add: element i sits at
    [i % 16, i // 16], replicated across the 8 Q7 cores (16*8=128 parts)."""
    t = flat.astype(np.int16).reshape(-1, 16).T  # [16, n/16]
    return np.ascontiguousarray(np.tile(t, (_NCORES, 1)))


def _prep_feature(idx, w_eff, off):
    """Host-side preprocessing for one feature: route values to vocab shards,
    build per-core packed index/weight/one-hot-column arrays and scatter maps.
    """
    seg = np.searchsorted(off, np.arange(_T, dtype=np.int64), side="right") - 1
    shard = idx // _VSH

    j_lists = [np.nonzero(shard == k)[0] for k in range(_NCORES)]
    nmax = max(len(j) for j in j_lists)
    vt = ((nmax + _CHUNK - 1) // _CHUNK) * _CHUNK  # padded values per core
    nblk = vt // _BLK

    # pick group size from worst-case span over all cores
    def nseg_max(gb):
        gv = gb * _BLK
        m = 0
        for j in j_lists:
            s = seg[j]
            s = np.append(s, np.full(vt - len(s), s[-1] if len(s) else 0))
            base = s[::gv]
            base_next = np.append(base[1:], s[-1] + 1)
            m = max(m, int((base_next - base).max()))
        return m

    gb = _pick_group_blks(nseg_max)
    gv = gb * _BLK
    ngrp = vt // gv

    per_core = []
    combine = []
    for k in range(_NCORES):
        j = j_lists[k]
        n = len(j)
        seg_k = seg[j]
        pad_seg = seg_k[-1] if n else 0
        seg_p = np.append(seg_k, np.full(vt - n, pad_seg))
        idx_p = np.append(idx[j] - k * _VSH, np.zeros(vt - n, np.int64))
        w_p = np.append(w_eff[j], np.zeros(vt - n, np.float32)).astype(np.float32)

        base = seg_p[::gv]  # [ngrp]
        base_next = np.append(base[1:], seg_p[-1] + 1)
        nseg = base_next - base
        assert int(nseg.max()) <= 127

        col = (seg_p - np.repeat(base, gv)).astype(np.float32)
        assert col.min() >= 0.0 and col.max() <= 127.0

        # scatter rows [128, ngrp]: psum row p of group g -> DRAM row
        p = np.arange(_BLK, dtype=np.int64)[:, None]
        rows = base[None, :] + p
        rows = np.where(p == nseg[None, :], _B + 1 + np.arange(ngrp)[None, :], rows)
        rows = np.where(p > nseg[None, :], _B, rows)  # trash row

        def pack(a):
            # value v = b*128 + p  ->  packed[p, b]
            return np.ascontiguousarray(a.reshape(-1, _BLK).T)

        per_core.append(
            {
                "gidx": _wrap16(idx_p),  # [128, vt/16] i16
                "col": pack(col).astype(np.float32),  # [128, nblk] f32
                "w": pack(w_p),  # [128, nblk] f32
                "srow": _wrap16(rows.T.reshape(-1)),  # [128, ngrp*8] i16
            }
        )
        combine.append(
            {
                "lo": int(seg_k[0]) if n else 0,
                "hi": (int(seg_k[-1]) + 1) if n else 0,
                "base_next": base_next,
            }
        )
    return dict(gb=gb, vt=vt, nblk=nblk, ngrp=ngrp, per_core=per_core,
                combine=combine)


_NC_CACHE = {}


def _build_nc(cfg0, cfg1):
    """Build (and cache) the SPMD Bass/Tile program; all data-dependence is
    carried by input tensors, so one program serves all 8 cores."""
    key = (cfg0["gb"], cfg0["vt"], cfg1["gb"], cfg1["vt"])
    if key in _NC_CACHE:
        return _NC_CACHE[key]

    from contextlib import ExitStack

    import concourse.tile as tile
    from concourse import bacc, mybir

    f32 = mybir.dt.float32
    i16 = mybir.dt.int16

    nc = bacc.Bacc(
        "TRN2", target_bir_lowering=False, debug=False, enable_asserts=False
    )

    feats = []
    for fi, (dd, cfg) in enumerate(((_D0, cfg0), (_D1, cfg1))):
        vt, nblk, ngrp = cfg["vt"], cfg["nblk"], cfg["ngrp"]
        feats.append(
            dict(
                fi=fi,
                D=dd,
                gb=cfg["gb"],
                ngrp=ngrp,
                nblk=nblk,
                vt=vt,
                tab=nc.dram_tensor(f"table{fi}", [_VSH, dd], f32, kind="ExternalInput"),
                gidx=nc.dram_tensor(f"gidx{fi}", [_BLK, vt // 16], i16, kind="ExternalInput"),
                col=nc.dram_tensor(f"col{fi}", [_BLK, nblk], f32, kind="ExternalInput"),
                w=nc.dram_tensor(f"w{fi}", [_BLK, nblk], f32, kind="ExternalInput"),
                srow=nc.dram_tensor(f"srow{fi}", [_BLK, ngrp * 8], i16, kind="ExternalInput"),
                out=nc.dram_tensor(f"out{fi}", [_B + 1 + ngrp, dd], f32, kind="ExternalOutput"),
            )
        )

    iota_np = np.tile(np.arange(_BLK, dtype=np.float32), (_BLK, 1))
    iota_dram = nc.inline_tensor(iota_np, name="iota128")

    with tile.TileContext(nc) as tc, ExitStack() as ctx:
        cpool = ctx.enter_context(tc.tile_pool(name="consts", bufs=1))
        iota_t = cpool.tile([_BLK, _BLK], f32, tag="iota")
        nc.sync.dma_start(iota_t[:], iota_dram.ap())

        for f in feats:
            D, gb, ngrp, nblk, vt = f["D"], f["gb"], f["ngrp"], f["nblk"], f["vt"]
            gpc = _CHUNK_BLKS // gb  # groups per chunk
            nchunk = vt // _CHUNK

            gidx_t = cpool.tile([_BLK, vt // 16], i16, tag=f"gidx{f['fi']}")
            col_t = cpool.tile([_BLK, nblk], f32, tag=f"col{f['fi']}")
            w_t = cpool.tile([_BLK, nblk], f32, tag=f"w{f['fi']}")
            srow_t = cpool.tile([_BLK, ngrp * 8], i16, tag=f"srow{f['fi']}")
            nc.sync.dma_start(gidx_t[:], f["gidx"].ap())
            nc.sync.dma_start(col_t[:], f["col"].ap())
            nc.sync.dma_start(w_t[:], f["w"].ap())
            nc.sync.dma_start(srow_t[:], f["srow"].ap())

            epool = ctx.enter_context(tc.tile_pool(name=f"emb{f['fi']}", bufs=4))
            spool = ctx.enter_context(tc.tile_pool(name=f"st{f['fi']}", bufs=4))
            mpool = ctx.enter_context(tc.tile_pool(name=f"m{f['fi']}", bufs=6))
            pspool = ctx.enter_context(
                tc.tile_pool(name=f"ps{f['fi']}", bufs=4, space="PSUM")
            )

            for c in range(nchunk):
                emb = epool.tile([_BLK, _CHUNK_BLKS * D], f32, tag="emb")
                nc.gpsimd.dma_gather(
                    out_ap=emb[:].rearrange("p (k d) -> p k d", d=D),
                    in_ap=f["tab"].ap(),
                    idxs_ap=gidx_t[:, c * (_CHUNK // 16) : (c + 1) * (_CHUNK // 16)],
                    num_idxs=_CHUNK,
                    num_idxs_reg=_CHUNK,
                    elem_size=D,
                    # >1024 descriptors don't fit one SWDGE packet; leaving
                    # single_packet on crashes the exec unit at this size
                    single_packet=False,
                )
                # fold the per-value weight into the gathered rows (in place)
                nc.vector.tensor_tensor(
                    out=emb[:].rearrange("p (k d) -> p k d", d=D),
                    in0=emb[:].rearrange("p (k d) -> p k d", d=D),
                    in1=w_t[:, c * _CHUNK_BLKS : (c + 1) * _CHUNK_BLKS]
                    .rearrange("p (k one) -> p k one", one=1)
                    .to_broadcast([_BLK, _CHUNK_BLKS, D]),
                    op=mybir.AluOpType.mult,
                )
                staged = spool.tile([_BLK, gpc * D], f32, tag="staged")
                for gi in range(gpc):
                    g = c * gpc + gi
                    # one-hot block-column matrices for this group in one op:
                    # m[j, (b, s)] = (col[j, b] == s)
                    m = mpool.tile([_BLK, gb * _BLK], f32, tag="m")
                    cb = c * _CHUNK_BLKS + gi * gb
                    nc.vector.tensor_tensor(
                        out=m[:].rearrange("p (k s) -> p k s", s=_BLK),
                        in0=iota_t[:]
                        .rearrange("p (one s) -> p one s", one=1)
                        .to_broadcast([_BLK, gb, _BLK]),
                        in1=col_t[:, cb : cb + gb]
                        .rearrange("p (k one) -> p k one", one=1)
                        .to_broadcast([_BLK, gb, _BLK]),
                        op=mybir.AluOpType.is_equal,
                    )
                    ps = pspool.tile([_BLK, D], f32, space="PSUM", tag="ps")
                    for b in range(gb):
                        blk = gi * gb + b
                        nc.tensor.matmul(
                            out=ps[:],
                            lhsT=m[:, b * _BLK : (b + 1) * _BLK],
                            rhs=emb[:, blk * D : (blk + 1) * D],
                            start=(b == 0),
                            stop=(b == gb - 1),
                        )
                    nc.scalar.copy(staged[:, gi * D : (gi + 1) * D], ps[:])

                nc.gpsimd.dma_scatter_add(
                    out_ap=f["out"].ap(),
                    in_ap=staged[:].rearrange("p (k d) -> p k d", d=D),
                    idxs_ap=srow_t[:, c * gpc * 8 : (c + 1) * gpc * 8],
                    num_idxs=gpc * _BLK,
                    num_idxs_reg=gpc * _BLK,
                    elem_size=D,
                )

    nc.compile()
    _NC_CACHE[key] = nc
    return nc


_LAST_RESULT = None  # BassKernelResults of the most recent run (for test.py)


def kernel(table0, table1, weights0, idx0, idx1, off0, off1, _trace=False):
    global _LAST_RESULT

    table0 = np.ascontiguousarray(np.asarray(table0, dtype=np.float32))
    table1 = np.ascontiguousarray(np.asarray(table1, dtype=np.float32))
    weights0 = np.asarray(weights0, dtype=np.float32)
    idx0 = np.asarray(idx0, dtype=np.int64)
    idx1 = np.asarray(idx1, dtype=np.int64)
    off0 = np.asarray(off0, dtype=np.int64)
    off1 = np.asarray(off1, dtype=np.int64)

    # Feature 1 is a mean-pool: fold 1/count into per-value weights.
    counts1 = (off1[1:] - off1[:-1]).astype(np.float64)
    seg1 = np.searchsorted(off1, np.arange(_T, dtype=np.int64), side="right") - 1
    w1 = (1.0 / np.maximum(counts1[seg1], 1.0)).astype(np.float32)

    cfg0 = _prep_feature(idx0, weights0, off0)
    cfg1 = _prep_feature(idx1, w1, off1)

    nc = _build_nc(cfg0, cfg1)

    in_maps = []
    for k in range(_NCORES):
        m = {
            "table0": table0[k * _VSH : (k + 1) * _VSH],
            "table1": table1[k * _VSH : (k + 1) * _VSH],
        }
        for fi, cfg in ((0, cfg0), (1, cfg1)):
            pc = cfg["per_core"][k]
            m[f"gidx{fi}"] = pc["gidx"]
            m[f"col{fi}"] = pc["col"]
            m[f"w{fi}"] = pc["w"]
            m[f"srow{fi}"] = pc["srow"]
        in_maps.append(m)

    from concourse.bass_utils import run_bass_kernel_spmd

    res = run_bass_kernel_spmd(
        nc, in_maps, core_ids=list(range(_NCORES)), trace=_trace
    )
    _LAST_RESULT = res

    out0 = np.zeros((_B, _D0), dtype=np.float32)
    out1 = np.zeros((_B, _D1), dtype=np.float32)
    for out, cfg, name in ((out0, cfg0, "out0"), (out1, cfg1, "out1")):
        ngrp = cfg["ngrp"]
        for k in range(_NCORES):
            buf = res.results[k][name]
            cmb = cfg["combine"][k]
            lo, hi = cmb["lo"], cmb["hi"]
            out[lo:hi] += buf[lo:hi]
            bn = cmb["base_next"]
            valid = bn < _B
            np.add.at(out, bn[valid], buf[_B + 1 : _B + 1 + ngrp][valid])
    return out0, out1
